# revision 10
# baseline (speedup 1.0000x reference)
"""Trainium2 Bass kernel for nn_DiscreteExactLoss (joint-entropy loss).

Reference computation:
    soft_assign[b, r, :] = [1 - a[b,r], a[b,r]]          (K=2, R=10)
    joint_p[b, s]  = prod_r soft_assign[b, r, s_r]       (s in [0, 1024))
    p_a            = mean_b joint_p                       [1024]
    out            = sum_s p_a * log2(p_a)               (scalar)

Device algorithm (per core, data-parallel over B across 8 cores):
    Accumulate MULTILINEAR MOMENTS m_T = sum_b prod_{r in T} a[b, r]
    factored over a 5+5 variable split: m_{T1 u T2} = sum_b MA[b,T1]*MC[b,T2],
    where MA/MC are the 32 subset-products of each 5-var half.  The
    32x32-per-sample outer product summed over b is a TensorEngine matmul
    with the contraction (128 samples) on the partition axis.

    Layout/scheduling choices (from trace analysis):
      * the host ships each core's shard in [p, r, c] layout (a pure
        relayout: sample b = p*128 + c on partition p, vars contiguous
        per-partition) so the two input DMAs move 2560B-contiguous runs and
        every DVE op runs on contiguous data in its fast 2x bf16 mode.
      * subset tables are m-major [128, 32, 128] so every build op writes
        long contiguous runs.  GpSimd is kept OFF the data path (it is
        ~3.5x slower than DVE and its SBUF traffic degrades DVE): DVE does
        the casts + all product levels, Scalar preps half C in parallel.
      * j=16 levels are split in cc-quarters so the 32 packed matmuls
        (LDWEIGHTS M=128 + MATMUL N=128, one PSUM [128,128] accumulator)
        start before the build fully ends.

    Host side: sum the 8 per-core [128,128] partials, fold the 4 diagonal
    c-blocks, apply the tiny Mobius transform (moments -> probabilities),
    then the p*log2(p) reduction (~30k flops, negligible).
"""

import math
import sys

import numpy as np

if "/opt/trn_rl_repo" not in sys.path:
    sys.path.insert(0, "/opt/trn_rl_repo")

B_FULL = 131072
R_FULL = 10
N_CORES = 8
B_LOC = B_FULL // N_CORES  # 16384
P = 128                    # SBUF partitions; samples per matmul chunk
C = B_LOC // P             # 128 sample-columns per partition
G = C // 4                 # 32 matmul groups (4 sample-columns each)
NQ = 4                     # j=16 level split into cc-quarters

_NC_CACHE = {}


def _build_module():
    if "nc" in _NC_CACHE:
        return _NC_CACHE["nc"]

    from concourse import bacc, bass, mybir, tile

    f32 = mybir.dt.float32
    bf16 = mybir.dt.bfloat16

    nc = bacc.Bacc(
        "TRN2", target_bir_lowering=False, debug=False, enable_partition_id=False
    )

    # per-core shard, host-relaid to [p, r, c]: sample b = p*C + c
    act_pl = nc.dram_tensor("act_pl", [P, R_FULL, C], f32, kind="ExternalInput")
    msum = nc.dram_tensor("msum", [P, P], f32, kind="ExternalOutput")

    act_ap = act_pl.ap()

    with tile.TileContext(nc) as tc:
        with (
            tc.tile_pool(name="a32", bufs=1) as a32_pool,
            tc.tile_pool(name="abf", bufs=1) as abf_pool,
            tc.tile_pool(name="mac", bufs=1) as mac_pool,
            tc.tile_pool(name="outp", bufs=1) as out_pool,
            tc.tile_pool(name="psum", bufs=1, space=bass.MemorySpace.PSUM) as psum_pool,
        ):
            # weights side: interleaved [p, g, m, c] so each matmul group's
            # 128 weight columns are one contiguous slab (single free dim,
            # FWL-eligible).  ifmap side: m-major [p, m, cc] for the fastest
            # fully-contiguous build (streamed operand tolerates 2 free dims).
            macA = mac_pool.tile([P, G, 32, 4], bf16, tag="macA")
            macC = mac_pool.tile([P, 32, C], bf16, tag="macC")

            # m=0 column := 1.0 (empty product); GpSimd, before data lands
            nc.gpsimd.memset(macA[:, :, 0:1, :], 1.0)
            nc.gpsimd.memset(macC[:, 0:1, :], 1.0)

            # raw f32 halves, split in cc-halves across both HWDGE queues so
            # half A (the DVE critical chain) lands as early as possible
            a32A = a32_pool.tile([P, 5, C], f32, tag="a32A")
            a32C = a32_pool.tile([P, 5, C], f32, tag="a32C")
            H = C // 2
            nc.sync.dma_start(out=a32A[:, :, 0:H], in_=act_ap[:, 0:5, 0:H])
            nc.scalar.dma_start(out=a32A[:, :, H:C], in_=act_ap[:, 0:5, H:C])
            nc.sync.dma_start(out=a32C[:, :, 0:H], in_=act_ap[:, 5:10, 0:H])
            nc.scalar.dma_start(out=a32C[:, :, H:C], in_=act_ap[:, 5:10, H:C])

            abfA = abf_pool.tile([P, 5, C], bf16, tag="abfA")
            abfC = abf_pool.tile([P, 5, C], bf16, tag="abfC")

            # half A prep on DVE (critical chain start); casts split per
            # DMA part so the first can start before the second part lands
            nc.vector.tensor_copy(abfA[:, :, 0:H], a32A[:, :, 0:H])
            nc.vector.tensor_copy(abfA[:, :, H:C], a32A[:, :, H:C])
            abfA_v = abfA.rearrange("p l (g c) -> p l g c", c=4)
            nc.vector.tensor_copy(
                macA[:, :, 1:2, :],
                abfA_v[:, 0:1, :, :].rearrange("p l g c -> p g l c"),
            )
            # half C prep on Scalar, in parallel with DVE's half-A work
            nc.scalar.copy(abfC[:, :, :], a32C[:, :, :])
            nc.scalar.copy(macC[:, 1:2, :], abfC[:, 0:1, :])

            # weights-side levels (interleaved layout)
            for lvl in range(1, 5):
                j = 1 << lvl
                a_b = abfA_v[:, lvl, :, :].unsqueeze(2).broadcast_to([P, G, j, 4])
                nc.vector.tensor_tensor(
                    macA[:, :, j:2 * j, :], macA[:, :, 0:j, :], a_b,
                    mybir.AluOpType.mult,
                )

            # ifmap-side levels (m-major layout), built in cc-halves so the
            # matmuls for the first half overlap the second half's build
            psum_acc = psum_pool.tile([P, P], f32)
            gph = G // 2
            for h in range(2):
                sl = slice(h * H, (h + 1) * H)
                for lvl in range(1, 5):
                    j = 1 << lvl
                    c_b = abfC[:, lvl:lvl + 1, sl].broadcast_to([P, j, H])
                    nc.vector.tensor_tensor(
                        macC[:, j:2 * j, sl], macC[:, 0:j, sl], c_b,
                        mybir.AluOpType.mult,
                    )
                for g in range(h * gph, (h + 1) * gph):
                    nc.tensor.matmul(
                        psum_acc[:, :],
                        macA[:, g, :, :],
                        macC[:, :, 4 * g:4 * g + 4],
                        start=(g == 0),
                        stop=(g == G - 1),
                    )

            out_sb = out_pool.tile([P, P], f32)
            nc.scalar.copy(out_sb[:, :], psum_acc[:, :])
            nc.scalar.dma_start(out=msum[:, :], in_=out_sb[:, :])

    nc.compile()
    _NC_CACHE["nc"] = nc
    return nc


def _ensure_ntff_hook():
    """The agent image's antenv package lacks axon_hooks; synthesize it so
    run_bass_kernel_spmd(trace=True) can find the NTFF profile hook."""
    import types

    try:
        from antenv.axon_hooks import get_axon_ntff_profile_hook  # noqa: F401
        return
    except ImportError:
        pass
    import antenv

    mod = types.ModuleType("antenv.axon_hooks")
    state = {"hook": None}
    mod.set_axon_ntff_profile_hook = lambda h: state.__setitem__("hook", h)
    mod.get_axon_ntff_profile_hook = lambda: state["hook"]
    antenv.axon_hooks = mod
    sys.modules["antenv.axon_hooks"] = mod

    try:
        from trn_agent_boot.trn_boot import _ntff_profile_via_ctypes

        hook = _ntff_profile_via_ctypes("/opt/axon/libaxon_pjrt.so")
        if hook is not None:
            mod.set_axon_ntff_profile_hook(hook)
    except Exception:
        pass


def _run_on_device(activity, trace=False):
    from concourse.bass_utils import run_bass_kernel_spmd

    if trace:
        _ensure_ntff_hook()
    nc = _build_module()
    shards = np.asarray(activity, dtype=np.float32).reshape(N_CORES, P, C, R_FULL)
    in_maps = [
        {"act_pl": np.ascontiguousarray(shards[i].transpose(0, 2, 1))}
        for i in range(N_CORES)
    ]
    res = run_bass_kernel_spmd(
        nc, in_maps, core_ids=list(range(N_CORES)), trace=trace
    )
    return res


def _finish_on_host(per_core_msums):
    # total [128,128] partial sums over all cores
    M2 = np.zeros((P, P), dtype=np.float64)
    for part in per_core_msums:
        M2 += part.astype(np.float64)
    # fold diagonal c-blocks: q = m*4 + c ; moments need c_i == c_j
    M2 = M2.reshape(32, 4, 32, 4)
    mom = sum(M2[:, c, :, c] for c in range(4)) / B_FULL  # [32A, 32C]
    m = mom.reshape(-1)  # T = mA*32 + mC (any consistent var labeling works)

    # Mobius transform per bit: p(bit=0) = m(without) - m(with)
    p = m.copy()
    idx = np.arange(1024)
    for bit in range(10):
        step = 1 << bit
        lo = idx[(idx & step) == 0]
        p[lo] = p[lo] - p[lo | step]

    p = p.astype(np.float32)
    p_safe = np.clip(p, 1e-12, None)
    log_k_p = np.log(p_safe) / math.log(2.0)
    joint_h = -np.sum(p * log_k_p)
    return np.array(-joint_h, dtype=np.float32)


def kernel(activity):
    res = _run_on_device(activity, trace=False)
    return _finish_on_host([r["msum"] for r in res.results])


def kernel_profiled(activity):
    """Like kernel() but with NTFF tracing; returns (output, exec_time_ns)."""
    res = _run_on_device(activity, trace=True)
    out = _finish_on_host([r["msum"] for r in res.results])
    return out, res.exec_time_ns
